# revision 41
# baseline (speedup 1.0000x reference)
"""LSH cosine-of-Hamming retrieval kernel for 8 trn2 NeuronCores.

Math: reference computes cos((pi/d) * hamming(u, v)) for binary LSH codes
u = (emb1 @ r.T > 0), v = (emb2 @ r.T > 0), d = 1024 bits.
With +/-1 sign codes s_u = 2u-1, s_v = 2v-1:
    hamming = (d - s_u . s_v) / 2
    cos((pi/d) * hamming) = sin((pi/2d) * s_u.s_v)
The kernel stores half-codes c = s/2 (+/-0.5, exact in fp8), so
out = sin((2*pi/d) * c_u.c_v).

Pipeline per core:
- Projection runs as a SINGLE fp16 matmul pass (full bf16-rate on the
  PE; fp16's ~11-bit mantissa gives rel err 1.28e-2, inside the 2e-2
  budget - the old bf16 hi/lo 3-pass scheme cost 3x the PE time and
  fp32/fp32r inputs cost 2x the input DMA, which gates the start).
- Binarization (PSUM f32 -> fp8 codes) is split across DVE
  (tensor_scalar is_gt -> +/-0.5) and ACT (Sign activation -> +/-1;
  sign and sin share the trig_and_small table so no table thrash).
  Only DVE/ACT can read PSUM (GPSIMD cannot). Scales stay uniform
  because chunk pairing puts ACT on v for bit-chunk pairs {0,1} and on
  u for pairs {2,3}: every bit's code product is 0.5, absorbed in the
  Sin scale (pi/1024).
- The fp8 DoubleRow code matmul runs in four n-passes of 512 v-rows:
  pass p only needs v row-chunk p converted, so it starts ~19us in,
  right after v chunk 0 and u chunk 0 convert; remaining projection
  chunks are interleaved between early pass-0 blocks, each a full
  conversion-latency ahead of its consumer. Single-bank psum tiles
  with a 4-deep pool keep Sin latency off the matmul critical path
  (a merged 2-block/1024-wide Sin variant measured ~3.5us slower).
- Output is stored as bf16 (halves store traffic; the f32 upcast
  happens on host), one 128KB DMA per (pass, m-block), biased 2:1
  toward the faster sync DMA queue.

Sharding (2x4 grid over 8 cores): core k computes the [2048, 2048] output
block for emb1 rows [(k//4)*2048...] x emb2 rows [(k%4)*2048...]; r is
replicated (collectives measure ~60us fixed cost under this runtime -
slower than the projection work they would save). Embeddings are passed
to the device pre-transposed (dim-major) - pure data layout prep.
"""

import sys

sys.path.insert(0, "/opt/trn_rl_repo")

import ml_dtypes
import numpy as np

import concourse.bacc as bacc
import concourse.tile as tile
from concourse import mybir
from concourse.bass_utils import run_bass_kernel_spmd

N1, N2, D, B = 4096, 8192, 128, 1024  # emb1 rows, emb2 rows, dim, num_bits
G1, G2 = 2, 4
M1, M2 = N1 // G1, N2 // G2  # 2048 x 2048 output block per core
KC = B // 128  # 8 bit-chunks of 128
RW = 512  # projection row-chunk width (fp32 moving-operand max)
NW = 512  # main matmul output tile width

_BUILD_CACHE = {}


def _dedupe_ldweights(nc):
    """Drop back-to-back InstLdweights with identical operands on the PE
    queue (consecutive main matmuls share a stationary operand when the
    s-loop is outer). Only sync-free loads are removed."""
    removed = 0
    for f in nc.m.functions:
        for bb in f.blocks:
            last_key = None
            for ins in list(bb.instructions):
                if type(ins).__name__ == "InstLdweights":
                    key = ins.concise()
                    if (
                        key == last_key
                        and not ins.has_wait()
                        and not ins.has_update()
                    ):
                        bb.instructions.remove(ins)
                        removed += 1
                    else:
                        last_key = key
    return removed


def _build(scale: float):
    if scale in _BUILD_CACHE:
        return _BUILD_CACHE[scale]
    nc = bacc.Bacc("TRN2", target_bir_lowering=False, debug=False)
    f32 = mybir.dt.float32
    f16 = mybir.dt.float16
    bf16 = mybir.dt.bfloat16
    fp8 = mybir.dt.float8e4

    e1d = nc.declare_dram_parameter("e1", [D, M1], f16, isOutput=False)
    e2d = nc.declare_dram_parameter("e2", [D, M2], f16, isOutput=False)
    rd = nc.declare_dram_parameter("r", [D, B], f16, isOutput=False)
    out = nc.declare_dram_parameter("out", [M1, M2], bf16, isOutput=True)

    with tile.TileContext(nc) as tc:
        with (
            tc.tile_pool(name="const", bufs=1) as const_pool,
            tc.tile_pool(name="codes", bufs=1) as code_pool,
            tc.tile_pool(name="outs", bufs=4) as out_pool,
            tc.tile_pool(name="psum", bufs=2, space="PSUM") as proj_pool,
            tc.tile_pool(name="mpsum", bufs=4, space="PSUM") as main_pool,
        ):
            # fp16 inputs are 1.25MB total; spread by arrival-order of
            # use over three queues (~90-190 GB/s each): everything lands
            # by ~11us. r rides sync alone so the projection's stationary
            # operands arrive first.
            r_sb = const_pool.tile([D, B], f16)
            e1_sb = const_pool.tile([D, M1], f16)
            e2_sb = const_pool.tile([D, M2], f16)

            warm = const_pool.tile([128, RW], bf16)
            nc.vector.memset(warm[:], 0.0)
            nc.sync.dma_start(r_sb[:], rd[:])
            nc.scalar.dma_start(e2_sb[:, 0:512], e2d[:, 0:512])
            nc.scalar.dma_start(e2_sb[:, 512:1024], e2d[:, 512:1024])
            nc.gpsimd.dma_start(e2_sb[:, 1024:1536], e2d[:, 1024:1536])
            nc.gpsimd.dma_start(e2_sb[:, 1536:2048], e2d[:, 1536:2048])
            nc.sync.dma_start(e1_sb[:, 0:512], e1d[:, 0:512])
            nc.sync.dma_start(e1_sb[:, 512:1024], e1d[:, 512:1024])
            nc.scalar.dma_start(e1_sb[:, 1024:1536], e1d[:, 1024:1536])
            nc.scalar.dma_start(e1_sb[:, 1536:2048], e1d[:, 1536:2048])

            ut = code_pool.tile([128, KC, M1], fp8)
            vt = code_pool.tile([128, KC, M2], fp8)

            # HAM warm-up: PE clock sits at 1.2 GHz until ~3.4us of
            # sustained activity; burn dummy matmuls while input DMAs fly.
            wps = proj_pool.tile([128, 2, RW], f32, name="pstile", tag="ps")
            for w in range(5):
                nc.tensor.matmul(
                    wps[:, w % 2, :], warm[:, 0:128], warm[:],
                    start=(w < 2), stop=(w >= 3),
                )

            # Projection group for one (tensor, row-chunk, c2): two fp16
            # matmuls fill a 2-bank psum tile [bits 128, 2, rows 512], then
            # one op makes fp8 codes with bits on partitions. `on_act`
            # selects the converter: ACT Sign (+/-1) vs DVE is_gt (+/-0.5).
            def proj_group(sb, dst, j, c2, on_act, w=RW):
                sl = slice(j * w, (j + 1) * w)
                ps = proj_pool.tile([128, 2, w], f32, name="pstile", tag="ps")
                for h in range(2):
                    cs = slice((2 * c2 + h) * 128, (2 * c2 + h + 1) * 128)
                    nc.tensor.matmul(
                        ps[:, h, :], r_sb[:, cs], sb[:, sl],
                        start=True, stop=True,
                    )
                dst_ap = dst[:, 2 * c2 : 2 * c2 + 2, sl]
                if on_act:
                    nc.scalar.activation(
                        dst_ap, ps[:], mybir.ActivationFunctionType.Sign
                    )
                else:
                    nc.vector.tensor_scalar(
                        dst_ap,
                        ps[:],
                        0.0,
                        0.5,
                        mybir.AluOpType.is_gt,
                        mybir.AluOpType.subtract,
                    )

            # chunk pairing: v pairs {0,1} on ACT, {2,3} on DVE; u is the
            # complement, so each bit's u*v code product is 0.5 (the two
            # converters also alternate within each row-chunk, which keeps
            # the 2-deep proj pool from serializing on one engine).
            def v_chunk(j):
                for c2 in range(KC // 2):
                    proj_group(e2_sb, vt, j, c2, on_act=(c2 < 2))

            def u_chunk(j):
                for c2 in range(KC // 2):
                    proj_group(e1_sb, ut, j, c2, on_act=(c2 >= 2))

            # Main code matmul runs in four n-passes of 512 v-rows; pass p
            # needs only v row-chunk p converted before it starts. Each
            # (pass, m-block) accumulates K=1024 into a single psum bank
            # via 4 DoubleRow matmuls, then one Sin ([128, 512] -> bf16)
            # and one 128KB store. The 4-deep main pool means a block's
            # allocation only waits on the Sin four blocks back - Sin
            # latency never paces the PE.
            def m_block(m, p):
                ot = out_pool.tile([128, NW], bf16)
                ms = slice(m * 128, (m + 1) * 128)
                ns = slice(p * NW, (p + 1) * NW)
                ps = main_pool.tile([128, NW], f32, name="mptile", tag="mps")
                for s in range(KC // 2):
                    nc.tensor.matmul(
                        ps[:],
                        ut[:, 2 * s : 2 * s + 2, ms],
                        vt[:, 2 * s : 2 * s + 2, ns],
                        start=(s == 0),
                        stop=(s == KC // 2 - 1),
                        perf_mode=mybir.MatmulPerfMode.DoubleRow,
                    )
                nc.scalar.activation(
                    ot[:],
                    ps[:],
                    mybir.ActivationFunctionType.Sin,
                    scale=scale * 2.0,
                )
                # 2:1 store bias toward the sync queue (the gpsimd DMA
                # queue measures ~half the bandwidth)
                (nc.gpsimd if (p * 16 + m) % 3 == 2 else nc.sync).dma_start(
                    out[ms, ns], ot[:]
                )

            # Interleave: pass 0 starts once v chunk 0 and u chunk 0 are
            # converted; u chunks 1-3 feed m-blocks 4k+ of every pass and
            # convert during pass 0's early blocks; v chunk p+1 projects
            # and converts a full pass ahead of its consumer.
            v_chunk(0)
            u_chunk(0)
            # pull half of v chunk 1 forward: the PE would otherwise idle
            # ~5us here waiting for v0/u0 conversions (long enough to
            # re-throttle the clock); these are real matmuls whose
            # conversions aren't needed until pass 1.
            proj_group(e2_sb, vt, 1, 0, on_act=True)
            proj_group(e2_sb, vt, 1, 1, on_act=True)
            m_block(0, 0)
            u_chunk(1)
            m_block(1, 0)
            m_block(2, 0)
            u_chunk(2)
            m_block(3, 0)
            m_block(4, 0)
            u_chunk(3)
            m_block(5, 0)
            m_block(6, 0)
            m_block(7, 0)
            m_block(8, 0)
            proj_group(e2_sb, vt, 1, 2, on_act=False)
            proj_group(e2_sb, vt, 1, 3, on_act=False)
            for m in range(9, M1 // 128):
                m_block(m, 0)
            for m in range(M1 // 128):
                if m == 2:
                    v_chunk(2)
                m_block(m, 1)
            for m in range(M1 // 128):
                if m == 2:
                    v_chunk(3)
                m_block(m, 2)
            for m in range(M1 // 128):
                m_block(m, 3)

    # Keep waits on the matmuls (not hoisted to ldweights) so redundant
    # weight loads stay sync-free and can be deduped away.
    nc.move_matmul_waits_to_ldweights = lambda: None
    nc.compile()
    _dedupe_ldweights(nc)
    _BUILD_CACHE[scale] = nc
    return nc


def _in_maps(emb1, emb2, r):
    rT = np.ascontiguousarray(r.T.astype(np.float16))
    e1T = np.ascontiguousarray(emb1.T.astype(np.float16))
    e2T = np.ascontiguousarray(emb2.T.astype(np.float16))
    maps = []
    for k in range(8):
        a, b = k // G2, k % G2
        s1 = slice(a * M1, (a + 1) * M1)
        s2 = slice(b * M2, (b + 1) * M2)
        maps.append(
            {
                "e1": np.ascontiguousarray(e1T[:, s1]),
                "e2": np.ascontiguousarray(e2T[:, s2]),
                "r": rT,
            }
        )
    return maps


def _install_profile_hook():
    """The agent image's antenv lacks axon_hooks; synthesize it so
    run_bass_kernel_spmd(trace=True) can reach the NTFF profiler."""
    import types

    if "antenv.axon_hooks" in sys.modules:
        return
    try:
        from trn_agent_boot.trn_boot import _ntff_profile_via_ctypes

        hook = _ntff_profile_via_ctypes("/opt/axon/libaxon_pjrt.so")
        mod = types.ModuleType("antenv.axon_hooks")
        mod.get_axon_ntff_profile_hook = lambda: hook
        sys.modules["antenv.axon_hooks"] = mod

        from concourse import bass_utils as _bu

        _orig_upload = _bu.upload_artifacts

        def _safe_upload(tmpdir):
            try:
                return _orig_upload(tmpdir)
            except Exception as e:  # no bucket access in this container
                return f"upload-skipped: {e}"

        _bu.upload_artifacts = _safe_upload
    except Exception:
        pass


def kernel(emb1, emb2, r, pi, _trace=False, _tmpdir=None):
    emb1 = np.asarray(emb1, dtype=np.float32)
    emb2 = np.asarray(emb2, dtype=np.float32)
    r = np.asarray(r, dtype=np.float32)
    # base scale pi/2048; each Sin divides by its block's code product
    scale = float(np.asarray(pi).reshape(-1)[0]) / (2.0 * B)

    nc = _build(scale)
    if _trace:
        _install_profile_hook()
    try:
        res = run_bass_kernel_spmd(
            nc, _in_maps(emb1, emb2, r), list(range(8)), trace=_trace, tmpdir=_tmpdir
        )
    except ModuleNotFoundError:
        res = run_bass_kernel_spmd(nc, _in_maps(emb1, emb2, r), list(range(8)))

    full = np.empty((N1, N2), dtype=np.float32)
    for k in range(8):
        a, b = k // G2, k % G2
        full[a * M1 : (a + 1) * M1, b * M2 : (b + 1) * M2] = res.results[k][
            "out"
        ].astype(np.float32)
    if _trace:
        kernel._last_exec_time_ns = res.exec_time_ns
    return full


# revision 42
# speedup vs baseline: 1.0077x; 1.0077x over previous
"""LSH cosine-of-Hamming retrieval kernel for 8 trn2 NeuronCores.

Math: reference computes cos((pi/d) * hamming(u, v)) for binary LSH codes
u = (emb1 @ r.T > 0), v = (emb2 @ r.T > 0), d = 1024 bits.
With +/-1 sign codes s_u = 2u-1, s_v = 2v-1:
    hamming = (d - s_u . s_v) / 2
    cos((pi/d) * hamming) = sin((pi/2d) * s_u.s_v)
The kernel stores half-codes c = s/2 (+/-0.5, exact in fp8), so
out = sin((2*pi/d) * c_u.c_v).

Pipeline per core:
- Projection runs as a SINGLE fp16 matmul pass (full bf16-rate on the
  PE; fp16's ~11-bit mantissa gives rel err 1.28e-2, inside the 2e-2
  budget - the old bf16 hi/lo 3-pass scheme cost 3x the PE time and
  fp32/fp32r inputs cost 2x the input DMA, which gates the start).
- Binarization (PSUM f32 -> fp8 codes) is split across DVE
  (tensor_scalar is_gt -> +/-0.5) and ACT (Sign activation -> +/-1;
  sign and sin share the trig_and_small table so no table thrash).
  Only DVE/ACT can read PSUM (GPSIMD cannot). Scales stay uniform
  because chunk pairing puts ACT on v for bit-chunk pairs {0,1} and on
  u for pairs {2,3}: every bit's code product is 0.5, absorbed in the
  Sin scale (pi/1024).
- The fp8 DoubleRow code matmul runs in four n-passes of 512 v-rows:
  pass p only needs v row-chunk p converted, so it starts ~19us in,
  right after v chunk 0 and u chunk 0 convert; remaining projection
  chunks are interleaved between early pass-0 blocks, each a full
  conversion-latency ahead of its consumer. Single-bank psum tiles
  with a 4-deep pool keep Sin latency off the matmul critical path
  (a merged 2-block/1024-wide Sin variant measured ~3.5us slower).
- Output is stored as bf16 (halves store traffic; the f32 upcast
  happens on host), one 128KB DMA per (pass, m-block), biased 2:1
  toward the faster sync DMA queue.

Sharding (2x4 grid over 8 cores): core k computes the [2048, 2048] output
block for emb1 rows [(k//4)*2048...] x emb2 rows [(k%4)*2048...]; r is
replicated (collectives measure ~60us fixed cost under this runtime -
slower than the projection work they would save). Embeddings are passed
to the device pre-transposed (dim-major) - pure data layout prep.
"""

import sys

sys.path.insert(0, "/opt/trn_rl_repo")

import ml_dtypes
import numpy as np

import concourse.bacc as bacc
import concourse.tile as tile
from concourse import mybir
from concourse.bass_utils import run_bass_kernel_spmd

N1, N2, D, B = 4096, 8192, 128, 1024  # emb1 rows, emb2 rows, dim, num_bits
G1, G2 = 2, 4
M1, M2 = N1 // G1, N2 // G2  # 2048 x 2048 output block per core
KC = B // 128  # 8 bit-chunks of 128
RW = 512  # projection row-chunk width (fp32 moving-operand max)
NW = 512  # main matmul output tile width

_BUILD_CACHE = {}


def _dedupe_ldweights(nc):
    """Drop back-to-back InstLdweights with identical operands on the PE
    queue (consecutive main matmuls share a stationary operand when the
    s-loop is outer). Only sync-free loads are removed."""
    removed = 0
    for f in nc.m.functions:
        for bb in f.blocks:
            last_key = None
            for ins in list(bb.instructions):
                if type(ins).__name__ == "InstLdweights":
                    key = ins.concise()
                    if (
                        key == last_key
                        and not ins.has_wait()
                        and not ins.has_update()
                    ):
                        bb.instructions.remove(ins)
                        removed += 1
                    else:
                        last_key = key
    return removed


def _build(scale: float):
    if scale in _BUILD_CACHE:
        return _BUILD_CACHE[scale]
    nc = bacc.Bacc("TRN2", target_bir_lowering=False, debug=False)
    f32 = mybir.dt.float32
    f16 = mybir.dt.float16
    bf16 = mybir.dt.bfloat16
    fp8 = mybir.dt.float8e4

    e1d = nc.declare_dram_parameter("e1", [D, M1], f16, isOutput=False)
    e2d = nc.declare_dram_parameter("e2", [D, M2], f16, isOutput=False)
    rd = nc.declare_dram_parameter("r", [D, B], f16, isOutput=False)
    out = nc.declare_dram_parameter("out", [M1, M2], bf16, isOutput=True)

    with tile.TileContext(nc) as tc:
        with (
            tc.tile_pool(name="const", bufs=1) as const_pool,
            tc.tile_pool(name="codes", bufs=1) as code_pool,
            tc.tile_pool(name="outs", bufs=8) as out_pool,
            tc.tile_pool(name="psum", bufs=2, space="PSUM") as proj_pool,
            tc.tile_pool(name="mpsum", bufs=4, space="PSUM") as main_pool,
        ):
            # fp16 inputs are 1.25MB total; spread by arrival-order of
            # use over three queues (~90-190 GB/s each): everything lands
            # by ~11us. r rides sync alone so the projection's stationary
            # operands arrive first.
            r_sb = const_pool.tile([D, B], f16)
            e1_sb = const_pool.tile([D, M1], f16)
            e2_sb = const_pool.tile([D, M2], f16)

            warm = const_pool.tile([128, RW], bf16)
            nc.vector.memset(warm[:], 0.0)
            nc.sync.dma_start(r_sb[:], rd[:])
            nc.scalar.dma_start(e2_sb[:, 0:512], e2d[:, 0:512])
            nc.scalar.dma_start(e2_sb[:, 512:1024], e2d[:, 512:1024])
            nc.gpsimd.dma_start(e2_sb[:, 1024:1536], e2d[:, 1024:1536])
            nc.gpsimd.dma_start(e2_sb[:, 1536:2048], e2d[:, 1536:2048])
            nc.sync.dma_start(e1_sb[:, 0:512], e1d[:, 0:512])
            nc.sync.dma_start(e1_sb[:, 512:1024], e1d[:, 512:1024])
            nc.scalar.dma_start(e1_sb[:, 1024:1536], e1d[:, 1024:1536])
            nc.scalar.dma_start(e1_sb[:, 1536:2048], e1d[:, 1536:2048])

            ut = code_pool.tile([128, KC, M1], fp8)
            vt = code_pool.tile([128, KC, M2], fp8)

            # HAM warm-up: PE clock sits at 1.2 GHz until ~3.4us of
            # sustained activity; burn dummy matmuls while input DMAs fly.
            wps = proj_pool.tile([128, 2, RW], f32, name="pstile", tag="ps")
            for w in range(5):
                nc.tensor.matmul(
                    wps[:, w % 2, :], warm[:, 0:128], warm[:],
                    start=(w < 2), stop=(w >= 3),
                )

            # Projection group for one (tensor, row-chunk, c2): two fp16
            # matmuls fill a 2-bank psum tile [bits 128, 2, rows 512], then
            # one op makes fp8 codes with bits on partitions. `on_act`
            # selects the converter: ACT Sign (+/-1) vs DVE is_gt (+/-0.5).
            def proj_group(sb, dst, j, c2, on_act, w=RW):
                sl = slice(j * w, (j + 1) * w)
                ps = proj_pool.tile([128, 2, w], f32, name="pstile", tag="ps")
                for h in range(2):
                    cs = slice((2 * c2 + h) * 128, (2 * c2 + h + 1) * 128)
                    nc.tensor.matmul(
                        ps[:, h, :], r_sb[:, cs], sb[:, sl],
                        start=True, stop=True,
                    )
                dst_ap = dst[:, 2 * c2 : 2 * c2 + 2, sl]
                if on_act:
                    nc.scalar.activation(
                        dst_ap, ps[:], mybir.ActivationFunctionType.Sign
                    )
                else:
                    nc.vector.tensor_scalar(
                        dst_ap,
                        ps[:],
                        0.0,
                        0.5,
                        mybir.AluOpType.is_gt,
                        mybir.AluOpType.subtract,
                    )

            # chunk pairing: v pairs {0,1} on ACT, {2,3} on DVE; u is the
            # complement, so each bit's u*v code product is 0.5 (the two
            # converters also alternate within each row-chunk, which keeps
            # the 2-deep proj pool from serializing on one engine).
            def v_chunk(j):
                for c2 in range(KC // 2):
                    proj_group(e2_sb, vt, j, c2, on_act=(c2 < 2))

            def u_chunk(j):
                for c2 in range(KC // 2):
                    proj_group(e1_sb, ut, j, c2, on_act=(c2 >= 2))

            # Main code matmul runs in four n-passes of 512 v-rows; pass p
            # needs only v row-chunk p converted before it starts. Each
            # (pass, m-block) accumulates K=1024 into a single psum bank
            # via 4 DoubleRow matmuls, then one Sin ([128, 512] -> bf16)
            # and one 128KB store. The 4-deep main pool means a block's
            # allocation only waits on the Sin four blocks back - Sin
            # latency never paces the PE.
            def m_block(m, p):
                ot = out_pool.tile([128, NW], bf16)
                ms = slice(m * 128, (m + 1) * 128)
                ns = slice(p * NW, (p + 1) * NW)
                ps = main_pool.tile([128, NW], f32, name="mptile", tag="mps")
                for s in range(KC // 2):
                    nc.tensor.matmul(
                        ps[:],
                        ut[:, 2 * s : 2 * s + 2, ms],
                        vt[:, 2 * s : 2 * s + 2, ns],
                        start=(s == 0),
                        stop=(s == KC // 2 - 1),
                        perf_mode=mybir.MatmulPerfMode.DoubleRow,
                    )
                nc.scalar.activation(
                    ot[:],
                    ps[:],
                    mybir.ActivationFunctionType.Sin,
                    scale=scale * 2.0,
                )
                # 2:1 store bias toward the sync queue (the gpsimd DMA
                # queue measures ~half the bandwidth)
                (nc.gpsimd if (p * 16 + m) % 3 == 2 else nc.sync).dma_start(
                    out[ms, ns], ot[:]
                )

            # Interleave: pass 0 starts once v chunk 0 and u chunk 0 are
            # converted; u chunks 1-3 feed m-blocks 4k+ of every pass and
            # convert during pass 0's early blocks; v chunk p+1 projects
            # and converts a full pass ahead of its consumer.
            v_chunk(0)
            u_chunk(0)
            # pull half of v chunk 1 forward: the PE would otherwise idle
            # ~5us here waiting for v0/u0 conversions (long enough to
            # re-throttle the clock); these are real matmuls whose
            # conversions aren't needed until pass 1.
            proj_group(e2_sb, vt, 1, 0, on_act=True)
            proj_group(e2_sb, vt, 1, 1, on_act=True)
            m_block(0, 0)
            u_chunk(1)
            m_block(1, 0)
            m_block(2, 0)
            u_chunk(2)
            m_block(3, 0)
            m_block(4, 0)
            u_chunk(3)
            m_block(5, 0)
            m_block(6, 0)
            m_block(7, 0)
            m_block(8, 0)
            proj_group(e2_sb, vt, 1, 2, on_act=False)
            proj_group(e2_sb, vt, 1, 3, on_act=False)
            for m in range(9, M1 // 128):
                m_block(m, 0)
            for m in range(M1 // 128):
                if m == 2:
                    v_chunk(2)
                m_block(m, 1)
            for m in range(M1 // 128):
                if m == 2:
                    v_chunk(3)
                m_block(m, 2)
            for m in range(M1 // 128):
                m_block(m, 3)

    # Keep waits on the matmuls (not hoisted to ldweights) so redundant
    # weight loads stay sync-free and can be deduped away.
    nc.move_matmul_waits_to_ldweights = lambda: None
    nc.compile()
    _dedupe_ldweights(nc)
    _BUILD_CACHE[scale] = nc
    return nc


def _in_maps(emb1, emb2, r):
    rT = np.ascontiguousarray(r.T.astype(np.float16))
    e1T = np.ascontiguousarray(emb1.T.astype(np.float16))
    e2T = np.ascontiguousarray(emb2.T.astype(np.float16))
    maps = []
    for k in range(8):
        a, b = k // G2, k % G2
        s1 = slice(a * M1, (a + 1) * M1)
        s2 = slice(b * M2, (b + 1) * M2)
        maps.append(
            {
                "e1": np.ascontiguousarray(e1T[:, s1]),
                "e2": np.ascontiguousarray(e2T[:, s2]),
                "r": rT,
            }
        )
    return maps


def _install_profile_hook():
    """The agent image's antenv lacks axon_hooks; synthesize it so
    run_bass_kernel_spmd(trace=True) can reach the NTFF profiler."""
    import types

    if "antenv.axon_hooks" in sys.modules:
        return
    try:
        from trn_agent_boot.trn_boot import _ntff_profile_via_ctypes

        hook = _ntff_profile_via_ctypes("/opt/axon/libaxon_pjrt.so")
        mod = types.ModuleType("antenv.axon_hooks")
        mod.get_axon_ntff_profile_hook = lambda: hook
        sys.modules["antenv.axon_hooks"] = mod

        from concourse import bass_utils as _bu

        _orig_upload = _bu.upload_artifacts

        def _safe_upload(tmpdir):
            try:
                return _orig_upload(tmpdir)
            except Exception as e:  # no bucket access in this container
                return f"upload-skipped: {e}"

        _bu.upload_artifacts = _safe_upload
    except Exception:
        pass


def kernel(emb1, emb2, r, pi, _trace=False, _tmpdir=None):
    emb1 = np.asarray(emb1, dtype=np.float32)
    emb2 = np.asarray(emb2, dtype=np.float32)
    r = np.asarray(r, dtype=np.float32)
    # base scale pi/2048; each Sin divides by its block's code product
    scale = float(np.asarray(pi).reshape(-1)[0]) / (2.0 * B)

    nc = _build(scale)
    if _trace:
        _install_profile_hook()
    try:
        res = run_bass_kernel_spmd(
            nc, _in_maps(emb1, emb2, r), list(range(8)), trace=_trace, tmpdir=_tmpdir
        )
    except ModuleNotFoundError:
        res = run_bass_kernel_spmd(nc, _in_maps(emb1, emb2, r), list(range(8)))

    full = np.empty((N1, N2), dtype=np.float32)
    for k in range(8):
        a, b = k // G2, k % G2
        full[a * M1 : (a + 1) * M1, b * M2 : (b + 1) * M2] = res.results[k][
            "out"
        ].astype(np.float32)
    if _trace:
        kernel._last_exec_time_ns = res.exec_time_ns
    return full
